# revision 13
# baseline (speedup 1.0000x reference)
"""Causal self-attention (B=4, T=2048, C=1024, H=16, D=64) on 8 trn2 cores.

Sharding: tensor-parallel over heads — 2 heads per core. Each core computes
qkv for its heads (x replicated, w_qkv column-sliced), RoPE, causal attention,
and a partial output projection (w_proj row-sliced); the host sums the 8
partial projections.

Device layout notes:
  - Everything "transposed": host supplies xT [C, B*T] so the contraction dim
    of every matmul sits on SBUF partitions with contiguous DMA lines.
  - q/k RoPE: rotate_half is a matmul against a +-1 permutation matrix (PE),
    then q' = q*cos + rot(q)*sin elementwise (DVE) against host-built tables.
  - Attention computed in "S^T" layout [j, i]: softmax denominator comes from
    an appended ones-column in V (so PV matmul emits sum(exp) as row 64);
    causal masking via gpsimd.affine_select on exp(S^T) tiles; no max-subtract
    (scores are bounded, exp cannot overflow in fp32).
  - matmul dtypes: fp32r (full-rate fp32-reduced) for qkv, bf16 for the
    attention/proj matmuls (fp32 PSUM accumulation throughout).
"""
import numpy as np
import ml_dtypes

import concourse.bass as bass
import concourse.mybir as mybir
import concourse.tile as tile

B, T, C = 4, 2048, 1024
H, D = 16, 64
NCORES = 8
HPC = H // NCORES          # heads per core = 2
N = B * T                  # 8192 rows
P = 128
FD = 512                   # i-tile / free-dim tile
KO = C // P                # 8 k-chunks for qkv
NT = N // FD               # 16 row tiles
ROPE_BASE = 10000.0

F32 = mybir.dt.float32
F32R = mybir.dt.float32r
BF16 = mybir.dt.bfloat16
AF = mybir.ActivationFunctionType


def build_nc():
    nc = bass.Bass()
    xT = nc.dram_tensor("xT", [C, N], F32R, kind="ExternalInput")
    w_all = nc.dram_tensor("w_all", [C, 3 * P], F32R, kind="ExternalInput")
    wp = nc.dram_tensor("wp", [P, C], BF16, kind="ExternalInput")
    cosb = nc.dram_tensor("cosb", [P, T], BF16, kind="ExternalInput")
    sinb = nc.dram_tensor("sinb", [P, T], BF16, kind="ExternalInput")
    rmat = nc.dram_tensor("rmat", [P, P], BF16, kind="ExternalInput")
    out = nc.dram_tensor("out", [N, C], F32, kind="ExternalOutput")

    xT_r = xT.rearrange("(ko p) n -> p ko n", p=P)          # [128, 8, N]
    w_r = w_all.rearrange("(ko p) m -> p ko m", p=P)        # [128, 8, 384]

    with tile.TileContext(nc) as tc:
        with tc.tile_pool(name="persist", bufs=1) as persist, \
             tc.tile_pool(name="work", bufs=3) as work, \
             tc.tile_pool(name="dstage", bufs=1, space="DRAM") as dstage:
            vt_dram = dstage.tile([P, N], BF16, tag="vt")

            # --- persistent tensors ---
            from concourse.library_config import all_libraries
            attn_lib = next(l for l in all_libraries if l.name == "attn")
            nc.gpsimd.load_library(attn_lib)

            w_sb = persist.tile([P, KO, 3 * P], F32R, tag="w_sb")
            nc.sync.dma_start(w_sb, w_r[:, :, :])
            wp_sb = persist.tile([P, C], BF16, tag="wp_sb")
            nc.sync.dma_start(wp_sb, wp[:, :])
            cos_sb = persist.tile([P, T], BF16, tag="cos_sb")
            nc.sync.dma_start(cos_sb, cosb[:, :])
            sin_sb = persist.tile([P, T], BF16, tag="sin_sb")
            nc.sync.dma_start(sin_sb, sinb[:, :])
            r_sb = persist.tile([P, P], BF16, tag="r_sb")
            nc.sync.dma_start(r_sb, rmat[:, :])

            qT = persist.tile([P, N], BF16, tag="qT")       # roped q, heads on partitions
            kT = persist.tile([P, N], BF16, tag="kT")
            yT = persist.tile([P, N], BF16, tag="yT")       # attention out (pre-proj)

            # =================== phase 1: qkv + rope ===================
            with tc.tile_pool(name="ps_qkv", bufs=2, space="PSUM") as ps_qkv, \
                 tc.tile_pool(name="ps_rot", bufs=1, space="PSUM") as ps_rot:
                for nt in range(NT):
                    n0 = nt * FD
                    t0 = n0 % T
                    x_sb = work.tile([P, KO, FD], F32R, tag="x_sb")
                    nc.sync.dma_start(x_sb, xT_r[:, :, n0:n0 + FD])

                    ps = {}
                    for im, m in enumerate(("q", "k", "v")):
                        pt = ps_qkv.tile([P, FD], F32, tag=f"ps_{m}")
                        for ko in range(KO):
                            nc.tensor.matmul(
                                pt, w_sb[:, ko, im * P:(im + 1) * P],
                                x_sb[:, ko, :],
                                start=(ko == 0), stop=(ko == KO - 1))
                        ps[m] = pt

                    # v: straight copy to DRAM staging (for transpose reload)
                    v_bf = work.tile([P, FD], BF16, tag="v_bf")
                    nc.scalar.activation(v_bf[:], ps["v"][:], AF.Copy)
                    nc.sync.dma_start(vt_dram[:, n0:n0 + FD], v_bf[:])

                    # q, k: rope
                    for m, dst in (("q", qT), ("k", kT)):
                        raw = work.tile([P, FD], BF16, tag=f"raw_{m}")
                        nc.vector.tensor_copy(raw[:], ps[m][:])
                        rot = ps_rot.tile([P, FD], F32, tag=f"rot_{m}")
                        nc.tensor.matmul(rot, r_sb, raw, start=True, stop=True)
                        t1 = work.tile([P, FD], BF16, tag=f"t1_{m}")
                        nc.vector.tensor_mul(t1[:], raw[:], cos_sb[:, t0:t0 + FD])
                        t2 = work.tile([P, FD], BF16, tag=f"t2_{m}")
                        nc.vector.tensor_mul(t2[:], rot[:], sin_sb[:, t0:t0 + FD])
                        nc.vector.tensor_add(dst[:, n0:n0 + FD], t1[:], t2[:])

            # =================== phase 2: attention ===================
            JB = 2  # j-chunks per ST/exp batch
            with tc.tile_pool(name="ps_att", bufs=1, space="PSUM") as ps_att:
                for b in range(B):
                    vsb = []
                    for h in range(HPC):
                        # dma_start_transpose corrupts non-contiguous dests on
                        # HW: land in a contiguous tile, then strided-copy.
                        vc = work.tile([P, T // P, D], BF16, tag=f"v_c{h}")
                        nc.sync.dma_start_transpose(
                            vc[:, :, :],
                            vt_dram[h * D:(h + 1) * D, b * T:(b + 1) * T])
                        vv = work.tile([P, T // P, D + 1], BF16, tag=f"v_sb{h}")
                        nc.any.memset(vv[:, :, D:D + 1], 1.0)
                        nc.vector.tensor_copy(vv[:, :, 0:D], vc[:, :, :])
                        vsb.append(vv)

                    for ti in range(T // FD):
                        i0 = ti * FD
                        njc = (ti + 1) * (FD // P)     # causal j-chunk count
                        py = [ps_att.tile([D + 1, FD], F32, tag=f"py{h}",
                                          name=f"py{h}")
                              for h in range(HPC)]
                        for jb in range(0, njc, JB):
                            nb = min(JB, njc - jb)
                            for h in range(HPC):
                                hs = h * D
                                st = ps_att.tile([P, JB * FD], F32, tag=f"st{h}")
                                for c in range(nb):
                                    j0 = (jb + c) * P
                                    nc.tensor.matmul(
                                        st[:, c * FD:(c + 1) * FD],
                                        kT[hs:hs + D, b * T + j0:b * T + j0 + P],
                                        qT[hs:hs + D, b * T + i0:b * T + i0 + FD],
                                        start=True, stop=True,
                                        tile_position=(hs, 0))
                                ex = work.tile([P, JB * FD], BF16, tag=f"ex{h}")
                                nc.scalar.activation(
                                    ex[:, 0:nb * FD], st[:, 0:nb * FD],
                                    AF.Exp, scale=float(D) ** -0.5)
                                for c in range(nb):
                                    jc = jb + c
                                    delta = jc * P - i0
                                    if delta > -P:  # diagonal-crossing chunk
                                        nc.gpsimd.affine_select(
                                            ex[:, c * FD:(c + 1) * FD],
                                            ex[:, c * FD:(c + 1) * FD],
                                            pattern=[[1, FD]],
                                            compare_op=mybir.AluOpType.is_ge,
                                            fill=0.0,
                                            base=-delta,
                                            channel_multiplier=-1)
                                    nc.tensor.matmul(
                                        py[h], vsb[h][:, jc, :],
                                        ex[:, c * FD:(c + 1) * FD],
                                        start=(jc == 0), stop=(jc == njc - 1))
                        # normalize: yT = pv / l  (l = row D of py).
                        # recip_fast + partition_broadcast require base
                        # partition 0 on HW, so shift l down first.
                        for h in range(HPC):
                            lrow = work.tile([P, FD], F32, tag="lrow")
                            nc.vector.tensor_copy(lrow[0:1, :], py[h][D:D + 1, :])
                            rc = work.tile([P, FD], F32, tag="rc")
                            nc.vector.reciprocal_approx_fast(
                                rc[0:1, :], lrow[0:1, :])
                            rb = work.tile([D, FD], F32, tag="rb")
                            nc.gpsimd.partition_broadcast(rb[:, :], rc[0:1, :])
                            nc.vector.tensor_mul(
                                yT[h * D:(h + 1) * D, b * T + i0:b * T + i0 + FD],
                                py[h][0:D, :], rb[:, :])

            # =================== phase 3: output projection ===================
            with tc.tile_pool(name="ps_o", bufs=2, space="PSUM") as ps_o:
                for mt in range(N // P):
                    o_sb = work.tile([P, C], F32, tag="o_sb")
                    for nh in range(C // FD):
                        po = ps_o.tile([P, FD], F32, tag="po")
                        nc.tensor.matmul(po, yT[:, mt * P:(mt + 1) * P],
                                         wp_sb[:, nh * FD:(nh + 1) * FD],
                                         start=True, stop=True)
                        if nh == 0:
                            nc.vector.tensor_copy(o_sb[:, nh * FD:(nh + 1) * FD], po[:])
                        else:
                            nc.scalar.activation(o_sb[:, nh * FD:(nh + 1) * FD],
                                                 po[:], AF.Copy)
                    nc.sync.dma_start(out[mt * P:(mt + 1) * P, :], o_sb[:])
    return nc


def split_multi_waits(nc):
    """walrus encodes only ONE sem wait per TPB instruction and does not
    auto-split. Hoist extra waits onto same-engine nops."""
    for blk in nc.main_func.blocks:
        new_insts = []
        for inst in blk.instructions:
            si = inst.sync_info
            if si is not None and si.on_wait and len(si.on_wait) > 1:
                for w in si.on_wait[:-1]:
                    nop = mybir.InstNoOp(
                        name=nc.get_next_instruction_name(), ins=[], outs=[])
                    nop.engine = inst.engine
                    nop.sync_info = mybir.SyncInfo(on_wait=[w], on_update=[])
                    nc.register_instruction(nop)
                    new_insts.append(nop)
                si.on_wait = si.on_wait[-1:]
            new_insts.append(inst)
        blk.instructions[:] = new_insts


def _rope_tables():
    inv_freq = 1.0 / (ROPE_BASE ** (np.arange(0, D, 2, dtype=np.float32) / D))
    t = np.arange(T, dtype=np.float32)
    freqs = np.outer(t, inv_freq)                       # [T, 32]
    emb = np.concatenate([freqs, freqs], -1)            # [T, 64]
    cos = np.cos(emb).T                                 # [64, T]
    sin = np.sin(emb).T
    cos2 = np.concatenate([cos, cos], 0)                # [128, T]
    sin2 = np.concatenate([sin, sin], 0)
    return cos2.astype(ml_dtypes.bfloat16), sin2.astype(ml_dtypes.bfloat16)


def _rot_lhsT():
    """lhsT[k, m] = R[m, k], rot(q)[m] = sum_k R[m,k] q[k], per 64-block."""
    r = np.zeros((D, D), np.float32)
    half = D // 2
    for m in range(half):
        r[m, m + half] = -1.0
    for m in range(half, D):
        r[m, m - half] = 1.0
    lhsT = r.T
    full = np.zeros((P, P), np.float32)
    full[:D, :D] = lhsT
    full[D:, D:] = lhsT
    return full.astype(ml_dtypes.bfloat16)


def make_in_maps(x, w_qkv, w_proj):
    x = np.asarray(x, np.float32)
    w_qkv = np.asarray(w_qkv, np.float32)
    w_proj = np.asarray(w_proj, np.float32)
    xT = np.ascontiguousarray(x.reshape(N, C).T)
    cos2, sin2 = _rope_tables()
    rmat = _rot_lhsT()
    in_maps = []
    for c in range(NCORES):
        h0 = c * HPC * D                      # first head's feature offset (128/core)
        cols = slice(h0, h0 + HPC * D)
        w_all = np.concatenate(
            [w_qkv[:, 0 * C:1 * C][:, cols],
             w_qkv[:, 1 * C:2 * C][:, cols],
             w_qkv[:, 2 * C:3 * C][:, cols]], axis=1)
        in_maps.append({
            "xT": xT,
            "w_all": np.ascontiguousarray(w_all),
            "wp": np.ascontiguousarray(w_proj[cols, :]).astype(ml_dtypes.bfloat16),
            "cosb": cos2,
            "sinb": sin2,
            "rmat": rmat,
        })
    return in_maps


_NC_CACHE = {}


def kernel(x, w_qkv, w_proj):
    from concourse.bass_utils import run_bass_kernel_spmd
    if "nc" not in _NC_CACHE:
        nc0 = build_nc()
        from concourse.library_overlay import lower_extended_insts
        lower_extended_insts(nc0)
        split_multi_waits(nc0)
        _NC_CACHE["nc"] = nc0
    nc = _NC_CACHE["nc"]
    in_maps = make_in_maps(x, w_qkv, w_proj)
    res = run_bass_kernel_spmd(nc, in_maps, list(range(NCORES)))
    acc = np.zeros((N, C), np.float64)
    for r in res.results:
        acc += r["out"].astype(np.float64)
    return acc.astype(np.float32).reshape(B, T, C)


# revision 16
# speedup vs baseline: 1.4533x; 1.4533x over previous
"""Causal self-attention (B=4, T=2048, C=1024, H=16, D=64) on 8 trn2 cores.

Sharding: tensor-parallel over heads — 2 heads per core. Each core computes
qkv for its heads (x replicated, w_qkv column-sliced), RoPE, causal attention,
and a partial output projection (w_proj row-sliced); the host sums the 8
partial projections.

Device layout notes:
  - Everything "transposed": host supplies xT [C, B*T] so the contraction dim
    of every matmul sits on SBUF partitions with contiguous DMA lines.
  - q/k RoPE: rotate_half is a matmul against a +-1 permutation matrix (PE),
    then q' = q*cos + rot(q)*sin elementwise (DVE) against host-built tables.
  - Attention computed in "S^T" layout [j, i]: softmax denominator comes from
    an appended ones-column in V (so PV matmul emits sum(exp) as row 64);
    causal masking via gpsimd.affine_select on exp(S^T) tiles; no max-subtract
    (scores are bounded, exp cannot overflow in fp32).
  - matmul dtypes: fp32r (full-rate fp32-reduced) for qkv, bf16 for the
    attention/proj matmuls (fp32 PSUM accumulation throughout).
"""
import numpy as np
import ml_dtypes

import concourse.bass as bass
import concourse.mybir as mybir
import concourse.tile as tile

B, T, C = 4, 2048, 1024
H, D = 16, 64
NCORES = 8
HPC = H // NCORES          # heads per core = 2
N = B * T                  # 8192 rows
P = 128
FD = 512                   # i-tile / free-dim tile
KO = C // P                # 8 k-chunks for qkv
NT = N // FD               # 16 row tiles
ROPE_BASE = 10000.0

F32 = mybir.dt.float32
F32R = mybir.dt.float32r
BF16 = mybir.dt.bfloat16
AF = mybir.ActivationFunctionType


def build_nc():
    nc = bass.Bass()
    xT = nc.dram_tensor("xT", [C, N], F32R, kind="ExternalInput")
    w_all = nc.dram_tensor("w_all", [C, 3 * P], F32R, kind="ExternalInput")
    wp = nc.dram_tensor("wp", [P, C], BF16, kind="ExternalInput")
    cosb = nc.dram_tensor("cosb", [P, T], BF16, kind="ExternalInput")
    sinb = nc.dram_tensor("sinb", [P, T], BF16, kind="ExternalInput")
    rmat = nc.dram_tensor("rmat", [P, P], BF16, kind="ExternalInput")
    out = nc.dram_tensor("out", [N, C], F32, kind="ExternalOutput")

    xT_r = xT.rearrange("(ko p) n -> p ko n", p=P)          # [128, 8, N]
    w_r = w_all.rearrange("(ko p) m -> p ko m", p=P)        # [128, 8, 384]

    with tile.TileContext(nc) as tc:
        with tc.tile_pool(name="persist", bufs=1) as persist, \
             tc.tile_pool(name="work", bufs=3) as work, \
             tc.tile_pool(name="dstage", bufs=1, space="DRAM") as dstage:
            vt_dram = [dstage.tile([P, T], BF16, tag=f"vt{b}", name=f"vt{b}")
                       for b in range(B)]

            # --- persistent tensors ---
            from concourse.library_config import all_libraries
            attn_lib = next(l for l in all_libraries if l.name == "attn")
            nc.gpsimd.load_library(attn_lib)

            w_sb = persist.tile([P, KO, 3 * P], F32R, tag="w_sb")
            nc.sync.dma_start(w_sb, w_r[:, :, :])
            wp_sb = persist.tile([P, C], BF16, tag="wp_sb")
            nc.sync.dma_start(wp_sb, wp[:, :])
            cos_sb = persist.tile([P, T], BF16, tag="cos_sb")
            nc.sync.dma_start(cos_sb, cosb[:, :])
            sin_sb = persist.tile([P, T], BF16, tag="sin_sb")
            nc.sync.dma_start(sin_sb, sinb[:, :])
            r_sb = persist.tile([P, P], BF16, tag="r_sb")
            nc.sync.dma_start(r_sb, rmat[:, :])

            qT = persist.tile([P, N], BF16, tag="qT")       # roped q, heads on partitions
            kT = persist.tile([P, N], BF16, tag="kT")
            yT = persist.tile([P, N], BF16, tag="yT")       # attention out (pre-proj)

            # =================== phase 1: qkv + rope ===================
            with tc.tile_pool(name="ps_qkv", bufs=2, space="PSUM") as ps_qkv, \
                 tc.tile_pool(name="ps_rot", bufs=1, space="PSUM") as ps_rot:
                for nt in range(NT):
                    n0 = nt * FD
                    t0 = n0 % T
                    x_sb = work.tile([P, KO, FD], F32R, tag="x_sb")
                    nc.sync.dma_start(x_sb, xT_r[:, :, n0:n0 + FD])

                    ps = {}
                    for im, m in enumerate(("q", "k", "v")):
                        pt = ps_qkv.tile([P, FD], F32, tag=f"ps_{m}")
                        for ko in range(KO):
                            nc.tensor.matmul(
                                pt, w_sb[:, ko, im * P:(im + 1) * P],
                                x_sb[:, ko, :],
                                start=(ko == 0), stop=(ko == KO - 1))
                        ps[m] = pt

                    # v: straight copy to DRAM staging (for transpose reload)
                    v_bf = work.tile([P, FD], BF16, tag="v_bf")
                    nc.vector.tensor_copy(v_bf[:], ps["v"][:])
                    nc.sync.dma_start(
                        vt_dram[n0 // T][:, n0 % T:n0 % T + FD], v_bf[:])

                    # q, k: rope
                    for m, dst in (("q", qT), ("k", kT)):
                        raw = work.tile([P, FD], BF16, tag=f"raw_{m}")
                        nc.scalar.activation(raw[:], ps[m][:], AF.Copy)
                        rot = ps_rot.tile([P, FD], F32, tag=f"rot_{m}")
                        nc.tensor.matmul(rot, r_sb, raw, start=True, stop=True)
                        t1 = work.tile([P, FD], BF16, tag=f"t1_{m}")
                        nc.vector.tensor_mul(t1[:], raw[:], cos_sb[:, t0:t0 + FD])
                        t2 = work.tile([P, FD], BF16, tag=f"t2_{m}")
                        nc.vector.tensor_mul(t2[:], rot[:], sin_sb[:, t0:t0 + FD])
                        nc.vector.tensor_add(dst[:, n0:n0 + FD], t1[:], t2[:])

            # ============ phase 2: attention + projection, per batch ============
            JB = 2  # j-chunks per ST/exp batch
            with tc.tile_pool(name="ps_att", bufs=2, space="PSUM") as ps_att, \
                 tc.tile_pool(name="ps_py", bufs=1, space="PSUM") as ps_py, \
                 tc.tile_pool(name="ps_o", bufs=2, space="PSUM") as ps_o:
                for b in range(B):
                    vsb = []
                    for h in range(HPC):
                        # dma_start_transpose corrupts non-contiguous dests on
                        # HW: land in a contiguous tile, then strided-copy.
                        vc = work.tile([P, T // P, D], BF16, tag=f"v_c{h}")
                        nc.sync.dma_start_transpose(
                            vc[:, :, :],
                            vt_dram[b][h * D:(h + 1) * D, :])
                        vv = work.tile([P, T // P, D + 1], BF16, tag=f"v_sb{h}")
                        nc.any.memset(vv[:, :, D:D + 1], 1.0)
                        nc.vector.tensor_copy(vv[:, :, 0:D], vc[:, :, :])
                        vsb.append(vv)

                    for ti in range(T // FD):
                        i0 = ti * FD
                        njc = (ti + 1) * (FD // P)     # causal j-chunk count
                        py = [ps_py.tile([D + 1, FD], F32, tag=f"py{h}",
                                         name=f"py{h}")
                              for h in range(HPC)]
                        for jb in range(0, njc, JB):
                            nb = min(JB, njc - jb)
                            for h in range(HPC):
                                hs = h * D
                                st = ps_att.tile([P, JB * FD], F32, tag="st",
                                                 name=f"st{h}")
                                for c in range(nb):
                                    j0 = (jb + c) * P
                                    nc.tensor.matmul(
                                        st[:, c * FD:(c + 1) * FD],
                                        kT[hs:hs + D, b * T + j0:b * T + j0 + P],
                                        qT[hs:hs + D, b * T + i0:b * T + i0 + FD],
                                        start=True, stop=True,
                                        tile_position=(hs, 0))
                                ex = work.tile([P, JB * FD], BF16, tag=f"ex{h}")
                                nc.scalar.activation(
                                    ex[:, 0:nb * FD], st[:, 0:nb * FD],
                                    AF.Exp, scale=float(D) ** -0.5)
                                for c in range(nb):
                                    jc = jb + c
                                    delta = jc * P - i0
                                    if delta > -P:  # diagonal-crossing chunk
                                        nc.gpsimd.affine_select(
                                            ex[:, c * FD:(c + 1) * FD],
                                            ex[:, c * FD:(c + 1) * FD],
                                            pattern=[[1, FD]],
                                            compare_op=mybir.AluOpType.is_ge,
                                            fill=0.0,
                                            base=-delta,
                                            channel_multiplier=-1)
                                    nc.tensor.matmul(
                                        py[h], vsb[h][:, jc, :],
                                        ex[:, c * FD:(c + 1) * FD],
                                        start=(jc == 0), stop=(jc == njc - 1))
                        # normalize: yT = pv / l  (l = row D of py).
                        # recip_fast + partition_broadcast require base
                        # partition 0 on HW, so shift l down first.
                        for h in range(HPC):
                            lrow = work.tile([P, FD], F32, tag="lrow")
                            nc.vector.tensor_copy(lrow[0:1, :], py[h][D:D + 1, :])
                            rc = work.tile([P, FD], F32, tag="rc")
                            nc.vector.reciprocal_approx_fast(
                                rc[0:1, :], lrow[0:1, :])
                            rb = work.tile([D, FD], F32, tag="rb")
                            nc.gpsimd.partition_broadcast(rb[:, :], rc[0:1, :])
                            nc.vector.tensor_mul(
                                yT[h * D:(h + 1) * D, b * T + i0:b * T + i0 + FD],
                                py[h][0:D, :], rb[:, :])

                    # projection for this batch's rows (overlaps next batch)
                    for mtl in range(T // P):
                        mt = b * (T // P) + mtl
                        o_sb = work.tile([P, C], F32, tag="o_sb")
                        for nh in range(C // FD):
                            po = ps_o.tile([P, FD], F32, tag="po")
                            nc.tensor.matmul(po, yT[:, mt * P:(mt + 1) * P],
                                             wp_sb[:, nh * FD:(nh + 1) * FD],
                                             start=True, stop=True)
                            nc.vector.tensor_copy(
                                o_sb[:, nh * FD:(nh + 1) * FD], po[:])
                        nc.sync.dma_start(out[mt * P:(mt + 1) * P, :], o_sb[:])
    return nc


def split_multi_waits(nc):
    """walrus encodes only ONE sem wait per TPB instruction and does not
    auto-split. Hoist extra waits onto same-engine nops."""
    for blk in nc.main_func.blocks:
        new_insts = []
        for inst in blk.instructions:
            si = inst.sync_info
            if si is not None and si.on_wait and len(si.on_wait) > 1:
                for w in si.on_wait[:-1]:
                    nop = mybir.InstNoOp(
                        name=nc.get_next_instruction_name(), ins=[], outs=[])
                    nop.engine = inst.engine
                    nop.sync_info = mybir.SyncInfo(on_wait=[w], on_update=[])
                    nc.register_instruction(nop)
                    new_insts.append(nop)
                si.on_wait = si.on_wait[-1:]
            new_insts.append(inst)
        blk.instructions[:] = new_insts


def _rope_tables():
    inv_freq = 1.0 / (ROPE_BASE ** (np.arange(0, D, 2, dtype=np.float32) / D))
    t = np.arange(T, dtype=np.float32)
    freqs = np.outer(t, inv_freq)                       # [T, 32]
    emb = np.concatenate([freqs, freqs], -1)            # [T, 64]
    cos = np.cos(emb).T                                 # [64, T]
    sin = np.sin(emb).T
    cos2 = np.concatenate([cos, cos], 0)                # [128, T]
    sin2 = np.concatenate([sin, sin], 0)
    return cos2.astype(ml_dtypes.bfloat16), sin2.astype(ml_dtypes.bfloat16)


def _rot_lhsT():
    """lhsT[k, m] = R[m, k], rot(q)[m] = sum_k R[m,k] q[k], per 64-block."""
    r = np.zeros((D, D), np.float32)
    half = D // 2
    for m in range(half):
        r[m, m + half] = -1.0
    for m in range(half, D):
        r[m, m - half] = 1.0
    lhsT = r.T
    full = np.zeros((P, P), np.float32)
    full[:D, :D] = lhsT
    full[D:, D:] = lhsT
    return full.astype(ml_dtypes.bfloat16)


def make_in_maps(x, w_qkv, w_proj):
    x = np.asarray(x, np.float32)
    w_qkv = np.asarray(w_qkv, np.float32)
    w_proj = np.asarray(w_proj, np.float32)
    xT = np.ascontiguousarray(x.reshape(N, C).T)
    cos2, sin2 = _rope_tables()
    rmat = _rot_lhsT()
    in_maps = []
    for c in range(NCORES):
        h0 = c * HPC * D                      # first head's feature offset (128/core)
        cols = slice(h0, h0 + HPC * D)
        w_all = np.concatenate(
            [w_qkv[:, 0 * C:1 * C][:, cols],
             w_qkv[:, 1 * C:2 * C][:, cols],
             w_qkv[:, 2 * C:3 * C][:, cols]], axis=1)
        in_maps.append({
            "xT": xT,
            "w_all": np.ascontiguousarray(w_all),
            "wp": np.ascontiguousarray(w_proj[cols, :]).astype(ml_dtypes.bfloat16),
            "cosb": cos2,
            "sinb": sin2,
            "rmat": rmat,
        })
    return in_maps


_NC_CACHE = {}


def kernel(x, w_qkv, w_proj):
    from concourse.bass_utils import run_bass_kernel_spmd
    if "nc" not in _NC_CACHE:
        nc0 = build_nc()
        from concourse.library_overlay import lower_extended_insts
        lower_extended_insts(nc0)
        split_multi_waits(nc0)
        _NC_CACHE["nc"] = nc0
    nc = _NC_CACHE["nc"]
    in_maps = make_in_maps(x, w_qkv, w_proj)
    res = run_bass_kernel_spmd(nc, in_maps, list(range(NCORES)))
    acc = np.zeros((N, C), np.float64)
    for r in res.results:
        acc += r["out"].astype(np.float64)
    return acc.astype(np.float32).reshape(B, T, C)
